# revision 20
# baseline (speedup 1.0000x reference)
"""HMM posterior kernel for Trainium2 (8 NeuronCores, SPMD data-parallel over batch).

Math: in the reference the forward/backward cumsum terms cancel, so the
pre-normalization log_gamma is independent of t:
    g_pre[b,k] = 2*ln_pi + ln_emis[b,T-1,k] + total[b,k] + (T-1)*ln_diag
    total[b,k] = -0.5*exp(-2*ls)*(S2[b] - 2*mu*S1[b] + T*mu^2) - T*(ls + C)
with S1 = sum_t x, S2 = sum_t x^2, C = 0.5*log(2*pi).  Collecting terms with
xl = x[T-1] (k-independent constants cancel in the logsumexp normalization):
    g_pre[b,k] ~ a[b]*Q[k] + s[b]*P[k] + R[k]
    Q = exp(-2*ls), P = Q*mu, a = -0.5*(S2 + xl^2), s = S1 + xl
    R = -0.5*(T+1)*Q*mu^2 + kc,  kc = -(T+1)*ls + 2*ln_pi + (T-1)*ln_diag
Output = (g_pre - logsumexp_k g_pre) broadcast over t -> [B, T, K].

Kernel structure (per core, BS=4 batch rows): everything is computed in a
128-partition *replicated* layout, so each row's final [128, m*K] tile is
already in the shape the broadcast output DMA needs.  Host-side prep is
layout-only (per-core batch slice, params tiled to [128, K] so their loads
are contiguous instead of 128 serialized stride-0 HBM reads, kc params
stacked [3, K], last obvs column extracted).  obvs is loaded [16, 4, 128]
(512 B descriptors); S1 comes from one DVE reduce, S2/2 from four DVE
scalar_tensor_tensor square+accum ops; one ones[16->128] PE matmul both
contracts the partials over partitions and re-broadcasts them; kc lands
broadcast in PSUM via one [3]-contraction PE matmul (emitted before the
obvs-gated stats matmul so PE FIFO order cannot stall it).  Per row, two
fused DVE stt ops + a PSUM add build g_pre, then logsumexp (DVE max, ACT
exp+accum/ln) and a 2-scalar normalize.  Rows alternate sync/gpsimd DMA
rings; row tiles carry MC=[2,4,4,4] materialized copies so output
descriptors are 4-8 KB, which streams the 16.8 MB of broadcast writes at
~385 GB/s with a balanced tail (memory regime; ~67 us vs 77 us for the
single-ring 2 KB-descriptor version).
"""

import numpy as np

B, T, K = 32, 2048, 512
NCORES = 8
BS = B // NCORES  # 4 batch rows per core
PW = 16           # t = p*128 + j layout: 16 partitions x 128 contiguous
RJ = T // 128     # 16 stride-0 repeats of a [128, K] tile per batch row
LOG_2PI = float(np.log(2.0 * np.pi))
C = 0.5 * LOG_2PI
SQH = float(np.sqrt(0.5))  # scale so ACT Square yields 0.5*x^2
C2 = 0.5 * (float(T) + 1.0)
SQC2 = float(np.sqrt(0.5 * (T + 1.0)))  # ACT Square yields c2*mu^2

_BUILT = {}


def _build_nc(split_waits=True):
    key = ("nc", split_waits)
    if key in _BUILT:
        return _BUILT[key]

    from concourse import bass, tile
    import concourse.mybir as mybir

    f32 = mybir.dt.float32
    AF = mybir.ActivationFunctionType
    ALU = mybir.AluOpType
    X = mybir.AxisListType.X

    nc = bass.Bass()
    obvs = nc.declare_dram_parameter("obvs", [BS, T], f32, isOutput=False)
    lsb = nc.declare_dram_parameter("lsb", [128, K], f32, isOutput=False)
    mub = nc.declare_dram_parameter("mub", [128, K], f32, isOutput=False)
    pd3f = nc.declare_dram_parameter("pd3f", [3, K], f32, isOutput=False)
    xlast = nc.declare_dram_parameter("xlast", [1, BS], f32, isOutput=False)
    out = nc.declare_dram_parameter("out", [BS, T, K], f32, isOutput=True)

    with tile.TileContext(nc) as tc:
        with (
            tc.tile_pool(name="sbuf", bufs=1) as pool,
            tc.tile_pool(name="psum", bufs=1, space="PSUM") as psum,
        ):
            # ---- tiny constants (DVE memsets, off critical path) ----
            ones128 = pool.tile([128, 128], f32)
            nc.vector.memset(ones128[:], 1.0)
            sp = pool.tile([PW, 16], f32)  # [S1(4), S2/2(4), xl(4), xl^2/2(4)]
            nc.vector.memset(sp[:], 0.0)
            c3 = pool.tile([1, 3], f32)
            nc.vector.memset(c3[0:1, 0:1], -(float(T) + 1.0))
            nc.vector.memset(c3[0:1, 1:2], 2.0)
            nc.vector.memset(c3[0:1, 2:3], float(T - 1))

            # coefT [3, 128]: rows of kc coefficients, built by a PE outer
            # product (DVE cannot memset partitions 1-2 directly).
            ps_c = psum.tile([3, 128], f32)
            nc.tensor.matmul(
                ps_c[:], lhsT=c3[:], rhs=ones128[0:1, :], start=True, stop=True
            )
            coefT = pool.tile([3, 128], f32)
            nc.vector.tensor_copy(coefT[:], ps_c[:])

            # ---- input DMAs.  sync ring: obvs (contiguous 512 B
            # descriptors) + 2 broadcast params; gpsimd SWDGE: xl + the
            # [3, K] param stack for the kc matmul.
            ob2 = pool.tile([PW, BS, 128], f32)
            nc.sync.dma_start(
                out=ob2[:], in_=obvs[:].rearrange("b (p j) -> p b j", j=128)
            )
            ls128 = pool.tile([128, K], f32)
            nc.sync.dma_start(out=ls128[:], in_=lsb[:])
            mu128 = pool.tile([128, K], f32)
            nc.scalar.dma_start(out=mu128[:], in_=mub[:])
            pd3 = pool.tile([3, K], f32)
            nc.gpsimd.dma_start(out=pd3[:], in_=pd3f[:])
            nc.gpsimd.dma_start(out=sp[0:1, 8:12], in_=xlast[:])

            # ACT: dummy to pull the ~1.3 us activation-table load off the
            # critical path (after the scalar-ring DMA dispatch).
            dummy = pool.tile([1, 1], f32)
            nc.scalar.activation(dummy[:], c3[0:1, 1:2], AF.Exp)

            # ---- batch stats in sp [16, 16]:
            # cols 0:4 = partial S1, 4:8 = partial S2/2 (sum over j of x^2/2),
            # 8:12 = xl (partition 0, via DMA), 12:16 = xl^2/2 (partition 0).
            nc.vector.reduce_sum(sp[:, 0:4].unsqueeze(2), ob2[:], axis=X)
            nc.scalar.activation(sp[0:1, 12:16], sp[0:1, 8:12], AF.Square, scale=SQH)
            sqs = pool.tile([PW, 128], f32)
            for b in range(BS):
                nc.vector.scalar_tensor_tensor(
                    out=sqs[:], in0=ob2[:, b, :], scalar=0.5, in1=ob2[:, b, :],
                    op0=ALU.mult, op1=ALU.mult,
                    accum_out=sp[:, 4 + b : 5 + b],
                )

            # kc broadcast in PSUM via one [3]-contraction PE matmul --
            # emitted BEFORE the stats matmul so the (early-ready) kc does
            # not queue behind the obvs-gated stats matmul on the PE.
            ps_kc = psum.tile([128, K], f32)
            nc.tensor.matmul(
                ps_kc[:], lhsT=coefT[:], rhs=pd3[:], start=True, stop=True
            )

            # ones[16->128] matmul: contracts the 16 stat partitions AND
            # broadcasts the result to all 128 partitions in one PE op.
            ps_s = psum.tile([128, 16], f32)
            nc.tensor.matmul(
                ps_s[:], lhsT=ones128[0:PW, :], rhs=sp[:], start=True, stop=True
            )
            st = pool.tile([128, 16], f32)
            nc.vector.tensor_copy(st[:], ps_s[:])

            # a = -(S2/2 + xl^2/2) ; s = S1 + xl   (both [128, BS])
            ab = pool.tile([128, BS], f32)
            nc.vector.scalar_tensor_tensor(
                out=ab[:], in0=st[:, 4:8], scalar=-1.0, in1=st[:, 12:16],
                op0=ALU.mult, op1=ALU.subtract,
            )
            sb = pool.tile([128, BS], f32)
            nc.vector.scalar_tensor_tensor(
                out=sb[:], in0=st[:, 0:4], scalar=1.0, in1=st[:, 8:12],
                op0=ALU.mult, op1=ALU.add,
            )

            # ---- param-only precompute (ACT): Q = exp(-2*ls) and
            # c2mu2 = 0.5*(T+1)*mu^2 via the Square pre-scale.
            Q = pool.tile([128, K], f32)
            nc.scalar.activation(Q[:], ls128[:], AF.Exp, scale=-2.0)
            c2mu2 = pool.tile([128, K], f32)
            nc.scalar.activation(c2mu2[:], mu128[:], AF.Square, scale=SQC2)

            # ---- per-row pipeline.  DVE: w = s_b*mu - c2mu2,
            # u = (w + a_b)*Q, g1 = u + kc, reduce_max, normalize;
            # ACT: exp (+accum) and ln; DMA rows alternate sync/gpsimd rings.
            u = [pool.tile([128, K], f32, tag=f"u{i}", name=f"u{i}") for i in range(2)]
            g1 = [pool.tile([128, K], f32, tag=f"g{i}", name=f"g{i}") for i in range(2)]
            e = pool.tile([128, K], f32)
            MC = [2, 4, 4, 4]  # copies of the row tile: desc size = MC*2 KB
            gf, negm, sm, nls = [], [], [], []
            for b in range(BS):
                gf.append(
                    pool.tile([128, MC[b] * K], f32, tag=f"gf{b}", name=f"gf{b}")
                )
                negm.append(pool.tile([128, 1], f32, tag=f"nm{b}", name=f"nm{b}"))
                sm.append(pool.tile([128, 1], f32, tag=f"sm{b}", name=f"sm{b}"))
                nls.append(pool.tile([128, 1], f32, tag=f"nl{b}", name=f"nl{b}"))

            for b in range(BS):
                ub, g1b = u[b % 2], g1[b % 2]
                nc.vector.scalar_tensor_tensor(
                    out=ub[:], in0=mu128[:], scalar=sb[:, b : b + 1], in1=c2mu2[:],
                    op0=ALU.mult, op1=ALU.subtract,
                )
                nc.vector.scalar_tensor_tensor(
                    out=ub[:], in0=ub[:], scalar=ab[:, b : b + 1], in1=Q[:],
                    op0=ALU.add, op1=ALU.mult,
                )
                nc.vector.tensor_add(g1b[:], ub[:], ps_kc[:])
                nc.vector.reduce_max(negm[b][:], g1b[:], axis=X, negate=True)
                nc.scalar.activation(
                    e[:], g1b[:], AF.Exp, bias=negm[b][:], accum_out=sm[b][:]
                )
                nc.scalar.activation(nls[b][:], sm[b][:], AF.Ln)
                nc.vector.tensor_scalar(
                    out=gf[b][:, 0:K], in0=g1b[:],
                    scalar1=negm[b][:], scalar2=nls[b][:],
                    op0=ALU.add, op1=ALU.subtract,
                )
                m = MC[b]
                w = K
                while w < m * K:  # doubling copies: [0:w] -> [w:2w]
                    nc.vector.tensor_copy(gf[b][:, w : 2 * w], gf[b][:, 0:w])
                    w *= 2
                eng = nc.sync if b in (0, 3) else nc.gpsimd
                eng.dma_start(
                    out=out[b].rearrange(
                        "(p j m) k -> p j (m k)", j=RJ // m, m=m
                    ),
                    in_=gf[b][:]
                    .unsqueeze(1)
                    .broadcast_to([128, RJ // m, m * K]),
                )

    if split_waits:
        _split_multi_waits(nc, mybir)
    _BUILT[key] = nc
    return nc


def _split_multi_waits(nc, mybir):
    """This walrus build allows at most ONE sync wait per instruction.  Split
    any instruction with N>1 waits into N-1 single-wait NoOps on the same
    engine (executed immediately before it by the same sequencer) plus the
    original instruction carrying the final wait."""
    for fn in nc.m.functions:
        for blk in fn.blocks:
            new_insts = []
            for inst in blk.instructions:
                si = inst.sync_info
                if si is not None and len(si.on_wait) > 1:
                    waits = list(si.on_wait)
                    for i, w in enumerate(waits[:-1]):
                        new_insts.append(
                            mybir.InstNoOp(
                                name=f"{inst.name}-sw{i}",
                                engine=inst.engine,
                                sync_info=mybir.SyncInfo(
                                    on_wait=[w], on_update=[]
                                ),
                                bass_nofuse=True,
                            )
                        )
                    inst.sync_info = mybir.SyncInfo(
                        on_wait=[waits[-1]], on_update=list(si.on_update)
                    )
                new_insts.append(inst)
            blk.instructions = new_insts


def _run(inputs, trace=False, trace_kwargs=None):
    from concourse.bass_utils import run_bass_kernel_spmd

    nc = _build_nc()
    obvs = np.ascontiguousarray(np.asarray(inputs["obvs"], dtype=np.float32))
    params = {
        name: np.ascontiguousarray(np.asarray(inputs[name], dtype=np.float32))
        for name in ("mu", "log_sigma", "ln_pi", "ln_diag")
    }
    lsb = np.ascontiguousarray(np.tile(params["log_sigma"][None, :], (128, 1)))
    mub = np.ascontiguousarray(np.tile(params["mu"][None, :], (128, 1)))
    pd3f = np.ascontiguousarray(
        np.stack([params["log_sigma"], params["ln_pi"], params["ln_diag"]])
    )
    in_maps = [
        {
            "obvs": obvs[c * BS : (c + 1) * BS],
            "lsb": lsb,
            "mub": mub,
            "pd3f": pd3f,
            "xlast": np.ascontiguousarray(
                obvs[c * BS : (c + 1) * BS, T - 1 : T].T
            ),
        }
        for c in range(NCORES)
    ]
    kw = {}
    if trace:
        kw["trace"] = True
        if trace_kwargs:
            kw["trace_kwargs"] = trace_kwargs
    res = run_bass_kernel_spmd(nc, in_maps, list(range(NCORES)), **kw)
    full = np.empty((B, T, K), dtype=np.float32)
    for c in range(NCORES):
        full[c * BS : (c + 1) * BS] = np.asarray(res.results[c]["out"])
    return full, res


def kernel(**inputs) -> np.ndarray:
    full, _ = _run(inputs, trace=False)
    return full


# revision 21
# speedup vs baseline: 1.0029x; 1.0029x over previous
"""HMM posterior kernel for Trainium2 (8 NeuronCores, SPMD data-parallel over batch).

Math: in the reference the forward/backward cumsum terms cancel, so the
pre-normalization log_gamma is independent of t:
    g_pre[b,k] = 2*ln_pi + ln_emis[b,T-1,k] + total[b,k] + (T-1)*ln_diag
    total[b,k] = -0.5*exp(-2*ls)*(S2[b] - 2*mu*S1[b] + T*mu^2) - T*(ls + C)
with S1 = sum_t x, S2 = sum_t x^2, C = 0.5*log(2*pi).  Collecting terms with
xl = x[T-1] (k-independent constants cancel in the logsumexp normalization):
    g_pre[b,k] ~ a[b]*Q[k] + s[b]*P[k] + R[k]
    Q = exp(-2*ls), P = Q*mu, a = -0.5*(S2 + xl^2), s = S1 + xl
    R = -0.5*(T+1)*Q*mu^2 + kc,  kc = -(T+1)*ls + 2*ln_pi + (T-1)*ln_diag
Output = (g_pre - logsumexp_k g_pre) broadcast over t -> [B, T, K].

Kernel structure (per core, BS=4 batch rows): everything is computed in a
128-partition *replicated* layout, so each row's final [128, m*K] tile is
already in the shape the broadcast output DMA needs.  Host-side prep is
layout-only (per-core batch slice, params tiled to [128, K] so their loads
are contiguous instead of 128 serialized stride-0 HBM reads, kc params
stacked [3, K], last obvs column extracted).  obvs is loaded [16, 4, 128]
(512 B descriptors); S1 comes from one DVE reduce, S2/2 from four DVE
scalar_tensor_tensor square+accum ops; one ones[16->128] PE matmul both
contracts the partials over partitions and re-broadcasts them; kc lands
broadcast in PSUM via one [3]-contraction PE matmul (emitted before the
obvs-gated stats matmul so PE FIFO order cannot stall it).  Per row, two
fused DVE stt ops + a PSUM add build g_pre, then logsumexp (DVE max, ACT
exp+accum/ln) and a 2-scalar normalize.  Rows alternate sync/gpsimd DMA
rings; row tiles carry MC=[2,4,4,4] materialized copies so output
descriptors are 4-8 KB, which streams the 16.8 MB of broadcast writes at
~385 GB/s with a balanced tail (memory regime; ~67 us vs 77 us for the
single-ring 2 KB-descriptor version).
"""

import numpy as np

B, T, K = 32, 2048, 512
NCORES = 8
BS = B // NCORES  # 4 batch rows per core
PW = 16           # t = p*128 + j layout: 16 partitions x 128 contiguous
RJ = T // 128     # 16 stride-0 repeats of a [128, K] tile per batch row
LOG_2PI = float(np.log(2.0 * np.pi))
C = 0.5 * LOG_2PI
SQH = float(np.sqrt(0.5))  # scale so ACT Square yields 0.5*x^2
C2 = 0.5 * (float(T) + 1.0)
SQC2 = float(np.sqrt(0.5 * (T + 1.0)))  # ACT Square yields c2*mu^2

_BUILT = {}


def _build_nc(split_waits=True):
    key = ("nc", split_waits)
    if key in _BUILT:
        return _BUILT[key]

    from concourse import bass, tile
    import concourse.mybir as mybir

    f32 = mybir.dt.float32
    AF = mybir.ActivationFunctionType
    ALU = mybir.AluOpType
    X = mybir.AxisListType.X

    nc = bass.Bass()
    obvs = nc.declare_dram_parameter("obvs", [BS, T], f32, isOutput=False)
    lsb = nc.declare_dram_parameter("lsb", [128, K], f32, isOutput=False)
    mub = nc.declare_dram_parameter("mub", [128, K], f32, isOutput=False)
    pd3f = nc.declare_dram_parameter("pd3f", [3, K], f32, isOutput=False)
    xlast = nc.declare_dram_parameter("xlast", [1, BS], f32, isOutput=False)
    out = nc.declare_dram_parameter("out", [BS, T, K], f32, isOutput=True)

    with tile.TileContext(nc) as tc:
        with (
            tc.tile_pool(name="sbuf", bufs=1) as pool,
            tc.tile_pool(name="psum", bufs=1, space="PSUM") as psum,
        ):
            # ---- tiny constants (DVE memsets, off critical path) ----
            ones128 = pool.tile([128, 128], f32)
            nc.vector.memset(ones128[:], 1.0)
            sp = pool.tile([PW, 16], f32)  # [S1(4), S2/2(4), xl(4), xl^2/2(4)]
            nc.vector.memset(sp[:], 0.0)
            c3 = pool.tile([1, 3], f32)
            nc.vector.memset(c3[0:1, 0:1], -(float(T) + 1.0))
            nc.vector.memset(c3[0:1, 1:2], 2.0)
            nc.vector.memset(c3[0:1, 2:3], float(T - 1))

            # coefT [3, 128]: rows of kc coefficients, built by a PE outer
            # product (DVE cannot memset partitions 1-2 directly).
            ps_c = psum.tile([3, 128], f32)
            nc.tensor.matmul(
                ps_c[:], lhsT=c3[:], rhs=ones128[0:1, :], start=True, stop=True
            )
            coefT = pool.tile([3, 128], f32)
            nc.vector.tensor_copy(coefT[:], ps_c[:])

            # ---- input DMAs.  sync ring: obvs (contiguous 512 B
            # descriptors) + 2 broadcast params; gpsimd SWDGE: xl + the
            # [3, K] param stack for the kc matmul.
            ob2 = pool.tile([PW, BS, 128], f32)
            nc.sync.dma_start(
                out=ob2[:], in_=obvs[:].rearrange("b (p j) -> p b j", j=128)
            )
            ls128 = pool.tile([128, K], f32)
            nc.sync.dma_start(out=ls128[:], in_=lsb[:])
            mu128 = pool.tile([128, K], f32)
            nc.scalar.dma_start(out=mu128[:], in_=mub[:])
            pd3 = pool.tile([3, K], f32)
            nc.gpsimd.dma_start(out=pd3[:], in_=pd3f[:])
            nc.gpsimd.dma_start(out=sp[0:1, 8:12], in_=xlast[:])

            # ACT: dummy to pull the ~1.3 us activation-table load off the
            # critical path (after the scalar-ring DMA dispatch).
            dummy = pool.tile([1, 1], f32)
            nc.scalar.activation(dummy[:], c3[0:1, 1:2], AF.Exp)

            # ---- batch stats in sp [16, 16]:
            # cols 0:4 = partial S1, 4:8 = partial S2/2 (sum over j of x^2/2),
            # 8:12 = xl (partition 0, via DMA), 12:16 = xl^2/2 (partition 0).
            nc.vector.reduce_sum(sp[:, 0:4].unsqueeze(2), ob2[:], axis=X)
            nc.scalar.activation(sp[0:1, 12:16], sp[0:1, 8:12], AF.Square, scale=SQH)
            sqs = pool.tile([PW, 128], f32)
            for b in range(BS):
                nc.vector.scalar_tensor_tensor(
                    out=sqs[:], in0=ob2[:, b, :], scalar=0.5, in1=ob2[:, b, :],
                    op0=ALU.mult, op1=ALU.mult,
                    accum_out=sp[:, 4 + b : 5 + b],
                )

            # kc broadcast in PSUM via one [3]-contraction PE matmul --
            # emitted BEFORE the stats matmul so the (early-ready) kc does
            # not queue behind the obvs-gated stats matmul on the PE.
            ps_kc = psum.tile([128, K], f32)
            nc.tensor.matmul(
                ps_kc[:], lhsT=coefT[:], rhs=pd3[:], start=True, stop=True
            )

            # ones[16->128] matmul: contracts the 16 stat partitions AND
            # broadcasts the result to all 128 partitions in one PE op.
            ps_s = psum.tile([128, 16], f32)
            nc.tensor.matmul(
                ps_s[:], lhsT=ones128[0:PW, :], rhs=sp[:], start=True, stop=True
            )
            st = pool.tile([128, 16], f32)
            nc.vector.tensor_copy(st[:], ps_s[:])

            # a = -(S2/2 + xl^2/2) ; s = S1 + xl   (both [128, BS])
            ab = pool.tile([128, BS], f32)
            nc.vector.scalar_tensor_tensor(
                out=ab[:], in0=st[:, 4:8], scalar=-1.0, in1=st[:, 12:16],
                op0=ALU.mult, op1=ALU.subtract,
            )
            sb = pool.tile([128, BS], f32)
            nc.vector.scalar_tensor_tensor(
                out=sb[:], in0=st[:, 0:4], scalar=1.0, in1=st[:, 8:12],
                op0=ALU.mult, op1=ALU.add,
            )

            # ---- param-only precompute (ACT): Q = exp(-2*ls) and
            # c2mu2 = 0.5*(T+1)*mu^2 via the Square pre-scale.
            Q = pool.tile([128, K], f32)
            nc.scalar.activation(Q[:], ls128[:], AF.Exp, scale=-2.0)
            c2mu2 = pool.tile([128, K], f32)
            nc.scalar.activation(c2mu2[:], mu128[:], AF.Square, scale=SQC2)

            # ---- per-row pipeline.  DVE: w = s_b*mu - c2mu2,
            # u = (w + a_b)*Q, g1 = u + kc, reduce_max, normalize;
            # ACT: exp (+accum) and ln; DMA rows alternate sync/gpsimd rings.
            u = [pool.tile([128, K], f32, tag=f"u{i}", name=f"u{i}") for i in range(2)]
            g1 = [pool.tile([128, K], f32, tag=f"g{i}", name=f"g{i}") for i in range(2)]
            e = pool.tile([128, K], f32)
            MC = [2, 4, 4, 4]  # copies of the row tile: desc size = MC*2 KB
            gf, negm, sm, nls = [], [], [], []
            for b in range(BS):
                gf.append(
                    pool.tile([128, MC[b] * K], f32, tag=f"gf{b}", name=f"gf{b}")
                )
                negm.append(pool.tile([128, 1], f32, tag=f"nm{b}", name=f"nm{b}"))
                sm.append(pool.tile([128, 1], f32, tag=f"sm{b}", name=f"sm{b}"))
                nls.append(pool.tile([128, 1], f32, tag=f"nl{b}", name=f"nl{b}"))

            for b in range(BS):
                ub, g1b = u[b % 2], g1[b % 2]
                nc.vector.scalar_tensor_tensor(
                    out=ub[:], in0=mu128[:], scalar=sb[:, b : b + 1], in1=c2mu2[:],
                    op0=ALU.mult, op1=ALU.subtract,
                )
                nc.vector.scalar_tensor_tensor(
                    out=ub[:], in0=ub[:], scalar=ab[:, b : b + 1], in1=Q[:],
                    op0=ALU.add, op1=ALU.mult,
                )
                nc.vector.tensor_add(g1b[:], ub[:], ps_kc[:])
                nc.vector.reduce_max(negm[b][:], g1b[:], axis=X, negate=True)
                nc.scalar.activation(
                    e[:], g1b[:], AF.Exp, bias=negm[b][:], accum_out=sm[b][:]
                )
                nc.scalar.activation(nls[b][:], sm[b][:], AF.Ln)
                nc.vector.tensor_scalar(
                    out=gf[b][:, 0:K], in0=g1b[:],
                    scalar1=negm[b][:], scalar2=nls[b][:],
                    op0=ALU.add, op1=ALU.subtract,
                )
                m = MC[b]
                w = K
                while w < m * K:  # doubling copies: [0:w] -> [w:2w]
                    nc.vector.tensor_copy(gf[b][:, w : 2 * w], gf[b][:, 0:w])
                    w *= 2
                eng = nc.sync if b % 2 == 0 else nc.gpsimd
                eng.dma_start(
                    out=out[b].rearrange(
                        "(p j m) k -> p j (m k)", j=RJ // m, m=m
                    ),
                    in_=gf[b][:]
                    .unsqueeze(1)
                    .broadcast_to([128, RJ // m, m * K]),
                )

    if split_waits:
        _split_multi_waits(nc, mybir)
    _BUILT[key] = nc
    return nc


def _split_multi_waits(nc, mybir):
    """This walrus build allows at most ONE sync wait per instruction.  Split
    any instruction with N>1 waits into N-1 single-wait NoOps on the same
    engine (executed immediately before it by the same sequencer) plus the
    original instruction carrying the final wait."""
    for fn in nc.m.functions:
        for blk in fn.blocks:
            new_insts = []
            for inst in blk.instructions:
                si = inst.sync_info
                if si is not None and len(si.on_wait) > 1:
                    waits = list(si.on_wait)
                    for i, w in enumerate(waits[:-1]):
                        new_insts.append(
                            mybir.InstNoOp(
                                name=f"{inst.name}-sw{i}",
                                engine=inst.engine,
                                sync_info=mybir.SyncInfo(
                                    on_wait=[w], on_update=[]
                                ),
                                bass_nofuse=True,
                            )
                        )
                    inst.sync_info = mybir.SyncInfo(
                        on_wait=[waits[-1]], on_update=list(si.on_update)
                    )
                new_insts.append(inst)
            blk.instructions = new_insts


def _run(inputs, trace=False, trace_kwargs=None):
    from concourse.bass_utils import run_bass_kernel_spmd

    nc = _build_nc()
    obvs = np.ascontiguousarray(np.asarray(inputs["obvs"], dtype=np.float32))
    params = {
        name: np.ascontiguousarray(np.asarray(inputs[name], dtype=np.float32))
        for name in ("mu", "log_sigma", "ln_pi", "ln_diag")
    }
    lsb = np.ascontiguousarray(np.tile(params["log_sigma"][None, :], (128, 1)))
    mub = np.ascontiguousarray(np.tile(params["mu"][None, :], (128, 1)))
    pd3f = np.ascontiguousarray(
        np.stack([params["log_sigma"], params["ln_pi"], params["ln_diag"]])
    )
    in_maps = [
        {
            "obvs": obvs[c * BS : (c + 1) * BS],
            "lsb": lsb,
            "mub": mub,
            "pd3f": pd3f,
            "xlast": np.ascontiguousarray(
                obvs[c * BS : (c + 1) * BS, T - 1 : T].T
            ),
        }
        for c in range(NCORES)
    ]
    kw = {}
    if trace:
        kw["trace"] = True
        if trace_kwargs:
            kw["trace_kwargs"] = trace_kwargs
    res = run_bass_kernel_spmd(nc, in_maps, list(range(NCORES)), **kw)
    full = np.empty((B, T, K), dtype=np.float32)
    for c in range(NCORES):
        full[c * BS : (c + 1) * BS] = np.asarray(res.results[c]["out"])
    return full, res


def kernel(**inputs) -> np.ndarray:
    full, _ = _run(inputs, trace=False)
    return full


# revision 22
# speedup vs baseline: 1.0068x; 1.0039x over previous
"""HMM posterior kernel for Trainium2 (8 NeuronCores, SPMD data-parallel over batch).

Math: in the reference the forward/backward cumsum terms cancel, so the
pre-normalization log_gamma is independent of t:
    g_pre[b,k] = 2*ln_pi + ln_emis[b,T-1,k] + total[b,k] + (T-1)*ln_diag
    total[b,k] = -0.5*exp(-2*ls)*(S2[b] - 2*mu*S1[b] + T*mu^2) - T*(ls + C)
with S1 = sum_t x, S2 = sum_t x^2, C = 0.5*log(2*pi).  Collecting terms with
xl = x[T-1] (k-independent constants cancel in the logsumexp normalization):
    g_pre[b,k] ~ a[b]*Q[k] + s[b]*P[k] + R[k]
    Q = exp(-2*ls), P = Q*mu, a = -0.5*(S2 + xl^2), s = S1 + xl
    R = -0.5*(T+1)*Q*mu^2 + kc,  kc = -(T+1)*ls + 2*ln_pi + (T-1)*ln_diag
Output = (g_pre - logsumexp_k g_pre) broadcast over t -> [B, T, K].

Kernel structure (per core, BS=4 batch rows): everything is computed in a
128-partition *replicated* layout, so each row's final [128, m*K] tile is
already in the shape the broadcast output DMA needs.  Host-side prep is
layout-only (per-core batch slice, params tiled to [128, K] so their loads
are contiguous instead of 128 serialized stride-0 HBM reads, kc params
stacked [3, K], last obvs column extracted).  obvs is loaded [16, 4, 128]
(512 B descriptors); S1 comes from one DVE reduce, S2/2 from four DVE
scalar_tensor_tensor square+accum ops; one ones[16->128] PE matmul both
contracts the partials over partitions and re-broadcasts them; kc lands
broadcast in PSUM via one [3]-contraction PE matmul (emitted before the
obvs-gated stats matmul so PE FIFO order cannot stall it).  Per row, two
fused DVE stt ops + a PSUM add build g_pre, then logsumexp (DVE max, ACT
exp+accum/ln) and a 2-scalar normalize.  Rows alternate sync/gpsimd DMA
rings; row tiles carry MC=[2,4,4,4] materialized copies so output
descriptors are 4-8 KB, which streams the 16.8 MB of broadcast writes at
~385 GB/s with a balanced tail (memory regime; ~67 us vs 77 us for the
single-ring 2 KB-descriptor version).
"""

import numpy as np

B, T, K = 32, 2048, 512
NCORES = 8
BS = B // NCORES  # 4 batch rows per core
PW = 16           # t = p*128 + j layout: 16 partitions x 128 contiguous
RJ = T // 128     # 16 stride-0 repeats of a [128, K] tile per batch row
LOG_2PI = float(np.log(2.0 * np.pi))
C = 0.5 * LOG_2PI
SQH = float(np.sqrt(0.5))  # scale so ACT Square yields 0.5*x^2
C2 = 0.5 * (float(T) + 1.0)
SQC2 = float(np.sqrt(0.5 * (T + 1.0)))  # ACT Square yields c2*mu^2

_BUILT = {}


def _build_nc(split_waits=True):
    key = ("nc", split_waits)
    if key in _BUILT:
        return _BUILT[key]

    from concourse import bass, tile
    import concourse.mybir as mybir

    f32 = mybir.dt.float32
    AF = mybir.ActivationFunctionType
    ALU = mybir.AluOpType
    X = mybir.AxisListType.X

    nc = bass.Bass()
    obvs = nc.declare_dram_parameter("obvs", [BS, T], f32, isOutput=False)
    lsb = nc.declare_dram_parameter("lsb", [128, K], f32, isOutput=False)
    mub = nc.declare_dram_parameter("mub", [128, K], f32, isOutput=False)
    pd3f = nc.declare_dram_parameter("pd3f", [3, K], f32, isOutput=False)
    xlast = nc.declare_dram_parameter("xlast", [1, BS], f32, isOutput=False)
    out = nc.declare_dram_parameter("out", [BS, T, K], f32, isOutput=True)

    with tile.TileContext(nc) as tc:
        with (
            tc.tile_pool(name="sbuf", bufs=1) as pool,
            tc.tile_pool(name="psum", bufs=1, space="PSUM") as psum,
        ):
            # ---- tiny constants (DVE memsets, off critical path) ----
            ones128 = pool.tile([128, 128], f32)
            nc.vector.memset(ones128[:], 1.0)
            sp = pool.tile([PW, 16], f32)  # [S1(4), S2/2(4), xl(4), xl^2/2(4)]
            nc.vector.memset(sp[:], 0.0)
            c3 = pool.tile([1, 3], f32)
            nc.vector.memset(c3[0:1, 0:1], -(float(T) + 1.0))
            nc.vector.memset(c3[0:1, 1:2], 2.0)
            nc.vector.memset(c3[0:1, 2:3], float(T - 1))

            # coefT [3, 128]: rows of kc coefficients, built by a PE outer
            # product (DVE cannot memset partitions 1-2 directly).
            ps_c = psum.tile([3, 128], f32)
            nc.tensor.matmul(
                ps_c[:], lhsT=c3[:], rhs=ones128[0:1, :], start=True, stop=True
            )
            coefT = pool.tile([3, 128], f32)
            nc.vector.tensor_copy(coefT[:], ps_c[:])

            # ---- input DMAs.  sync ring: obvs (contiguous 512 B
            # descriptors) + 2 broadcast params; gpsimd SWDGE: xl + the
            # [3, K] param stack for the kc matmul.
            ob2 = pool.tile([PW, BS, 128], f32)
            nc.sync.dma_start(
                out=ob2[:], in_=obvs[:].rearrange("b (p j) -> p b j", j=128)
            )
            ls128 = pool.tile([128, K], f32)
            nc.sync.dma_start(out=ls128[:], in_=lsb[:])
            mu128 = pool.tile([128, K], f32)
            nc.scalar.dma_start(out=mu128[:], in_=mub[:])
            pd3 = pool.tile([3, K], f32)
            nc.gpsimd.dma_start(out=pd3[:], in_=pd3f[:])
            nc.gpsimd.dma_start(out=sp[0:1, 8:12], in_=xlast[:])

            # ACT: dummy to pull the ~1.3 us activation-table load off the
            # critical path (after the scalar-ring DMA dispatch).
            dummy = pool.tile([1, 1], f32)
            nc.scalar.activation(dummy[:], c3[0:1, 1:2], AF.Exp)

            # ---- batch stats in sp [16, 16]:
            # cols 0:4 = partial S1, 4:8 = partial S2/2 (sum over j of x^2/2),
            # 8:12 = xl (partition 0, via DMA), 12:16 = xl^2/2 (partition 0).
            nc.vector.reduce_sum(sp[:, 0:4].unsqueeze(2), ob2[:], axis=X)
            nc.scalar.activation(sp[0:1, 12:16], sp[0:1, 8:12], AF.Square, scale=SQH)
            sqs = pool.tile([PW, 128], f32)
            for b in range(BS):
                nc.vector.scalar_tensor_tensor(
                    out=sqs[:], in0=ob2[:, b, :], scalar=0.5, in1=ob2[:, b, :],
                    op0=ALU.mult, op1=ALU.mult,
                    accum_out=sp[:, 4 + b : 5 + b],
                )

            # kc broadcast in PSUM via one [3]-contraction PE matmul --
            # emitted BEFORE the stats matmul so the (early-ready) kc does
            # not queue behind the obvs-gated stats matmul on the PE.
            ps_kc = psum.tile([128, K], f32)
            nc.tensor.matmul(
                ps_kc[:], lhsT=coefT[:], rhs=pd3[:], start=True, stop=True
            )

            # ones[16->128] matmul: contracts the 16 stat partitions AND
            # broadcasts the result to all 128 partitions in one PE op.
            ps_s = psum.tile([128, 16], f32)
            nc.tensor.matmul(
                ps_s[:], lhsT=ones128[0:PW, :], rhs=sp[:], start=True, stop=True
            )
            st = pool.tile([128, 16], f32)
            nc.vector.tensor_copy(st[:], ps_s[:])

            # a = -(S2/2 + xl^2/2) ; s = S1 + xl   (both [128, BS])
            ab = pool.tile([128, BS], f32)
            nc.vector.scalar_tensor_tensor(
                out=ab[:], in0=st[:, 4:8], scalar=-1.0, in1=st[:, 12:16],
                op0=ALU.mult, op1=ALU.subtract,
            )
            sb = pool.tile([128, BS], f32)
            nc.vector.scalar_tensor_tensor(
                out=sb[:], in0=st[:, 0:4], scalar=1.0, in1=st[:, 8:12],
                op0=ALU.mult, op1=ALU.add,
            )

            # ---- param-only precompute (ACT): Q = exp(-2*ls) and
            # c2mu2 = 0.5*(T+1)*mu^2 via the Square pre-scale.
            Q = pool.tile([128, K], f32)
            nc.scalar.activation(Q[:], ls128[:], AF.Exp, scale=-2.0)
            c2mu2 = pool.tile([128, K], f32)
            nc.scalar.activation(c2mu2[:], mu128[:], AF.Square, scale=SQC2)

            # ---- per-row pipeline.  DVE: w = s_b*mu - c2mu2,
            # u = (w + a_b)*Q, g1 = u + kc, reduce_max, normalize;
            # ACT: exp (+accum) and ln; DMA rows alternate sync/gpsimd rings.
            # Single shared u/g1 buffers: the WAR hazards force the Tile
            # scheduler to finish row b's chain before row b+1's w/u ops,
            # instead of interleaving them ahead of row 0's critical tail.
            u = [pool.tile([128, K], f32, tag="u0", name="u0")] * 2
            g1 = [pool.tile([128, K], f32, tag="g0", name="g0")] * 2
            e = pool.tile([128, K], f32)
            MC = [2, 4, 4, 4]  # copies of the row tile: desc size = MC*2 KB
            gf, negm, sm, nls = [], [], [], []
            for b in range(BS):
                gf.append(
                    pool.tile([128, MC[b] * K], f32, tag=f"gf{b}", name=f"gf{b}")
                )
                negm.append(pool.tile([128, 1], f32, tag=f"nm{b}", name=f"nm{b}"))
                sm.append(pool.tile([128, 1], f32, tag=f"sm{b}", name=f"sm{b}"))
                nls.append(pool.tile([128, 1], f32, tag=f"nl{b}", name=f"nl{b}"))

            for b in range(BS):
                ub, g1b = u[b % 2], g1[b % 2]
                nc.vector.scalar_tensor_tensor(
                    out=ub[:], in0=mu128[:], scalar=sb[:, b : b + 1], in1=c2mu2[:],
                    op0=ALU.mult, op1=ALU.subtract,
                )
                nc.vector.scalar_tensor_tensor(
                    out=ub[:], in0=ub[:], scalar=ab[:, b : b + 1], in1=Q[:],
                    op0=ALU.add, op1=ALU.mult,
                )
                nc.vector.tensor_add(g1b[:], ub[:], ps_kc[:])
                nc.vector.reduce_max(negm[b][:], g1b[:], axis=X, negate=True)
                nc.scalar.activation(
                    e[:], g1b[:], AF.Exp, bias=negm[b][:], accum_out=sm[b][:]
                )
                nc.scalar.activation(nls[b][:], sm[b][:], AF.Ln)
                nc.vector.tensor_scalar(
                    out=gf[b][:, 0:K], in0=g1b[:],
                    scalar1=negm[b][:], scalar2=nls[b][:],
                    op0=ALU.add, op1=ALU.subtract,
                )
                m = MC[b]
                w = K
                while w < m * K:  # doubling copies: [0:w] -> [w:2w]
                    nc.vector.tensor_copy(gf[b][:, w : 2 * w], gf[b][:, 0:w])
                    w *= 2
                eng = nc.sync if b % 2 == 0 else nc.gpsimd
                eng.dma_start(
                    out=out[b].rearrange(
                        "(p j m) k -> p j (m k)", j=RJ // m, m=m
                    ),
                    in_=gf[b][:]
                    .unsqueeze(1)
                    .broadcast_to([128, RJ // m, m * K]),
                )

    if split_waits:
        _split_multi_waits(nc, mybir)
    _BUILT[key] = nc
    return nc


def _split_multi_waits(nc, mybir):
    """This walrus build allows at most ONE sync wait per instruction.  Split
    any instruction with N>1 waits into N-1 single-wait NoOps on the same
    engine (executed immediately before it by the same sequencer) plus the
    original instruction carrying the final wait."""
    for fn in nc.m.functions:
        for blk in fn.blocks:
            new_insts = []
            for inst in blk.instructions:
                si = inst.sync_info
                if si is not None and len(si.on_wait) > 1:
                    waits = list(si.on_wait)
                    for i, w in enumerate(waits[:-1]):
                        new_insts.append(
                            mybir.InstNoOp(
                                name=f"{inst.name}-sw{i}",
                                engine=inst.engine,
                                sync_info=mybir.SyncInfo(
                                    on_wait=[w], on_update=[]
                                ),
                                bass_nofuse=True,
                            )
                        )
                    inst.sync_info = mybir.SyncInfo(
                        on_wait=[waits[-1]], on_update=list(si.on_update)
                    )
                new_insts.append(inst)
            blk.instructions = new_insts


def _run(inputs, trace=False, trace_kwargs=None):
    from concourse.bass_utils import run_bass_kernel_spmd

    nc = _build_nc()
    obvs = np.ascontiguousarray(np.asarray(inputs["obvs"], dtype=np.float32))
    params = {
        name: np.ascontiguousarray(np.asarray(inputs[name], dtype=np.float32))
        for name in ("mu", "log_sigma", "ln_pi", "ln_diag")
    }
    lsb = np.ascontiguousarray(np.tile(params["log_sigma"][None, :], (128, 1)))
    mub = np.ascontiguousarray(np.tile(params["mu"][None, :], (128, 1)))
    pd3f = np.ascontiguousarray(
        np.stack([params["log_sigma"], params["ln_pi"], params["ln_diag"]])
    )
    in_maps = [
        {
            "obvs": obvs[c * BS : (c + 1) * BS],
            "lsb": lsb,
            "mub": mub,
            "pd3f": pd3f,
            "xlast": np.ascontiguousarray(
                obvs[c * BS : (c + 1) * BS, T - 1 : T].T
            ),
        }
        for c in range(NCORES)
    ]
    kw = {}
    if trace:
        kw["trace"] = True
        if trace_kwargs:
            kw["trace_kwargs"] = trace_kwargs
    res = run_bass_kernel_spmd(nc, in_maps, list(range(NCORES)), **kw)
    full = np.empty((B, T, K), dtype=np.float32)
    for c in range(NCORES):
        full[c * BS : (c + 1) * BS] = np.asarray(res.results[c]["out"])
    return full, res


def kernel(**inputs) -> np.ndarray:
    full, _ = _run(inputs, trace=False)
    return full
